# revision 6
# baseline (speedup 1.0000x reference)
"""Trainium2 Bass kernel for the 3-layer GATv2 network (nn_GAT_35940286333219).

Sharding: nodes contiguously across 8 cores (2048 each); edges partitioned by
destination so segment-softmax/scatter-add stay local; per-layer AllGather of
the source-side transformed features XL = act @ Wl; per-edge source rows via
one dma_gather per (block, layer).

v3: C=256 (512B rows) so a single dma_gather fetches a whole block's edge rows
(48 SWDGE instructions instead of 864 indirect DMAs); per-edge logits via the
Lrelu activation (alpha=0.2) directly instead of the abs decomposition with
extra weight columns; all per-tile DVE element-wise work batched into
per-block ops with broadcast APs; the transposed one-hot M^T precomputed on
the host and DMA'd instead of per-tile PE transposes + PSUM round trips.
"""
import os
import numpy as np
import ml_dtypes

import concourse.bacc as bacc
import concourse.bass as bass
import concourse.mybir as mybir
import concourse.tile as tile
from concourse.bass_utils import run_bass_kernel_spmd
from concourse.masks import make_identity

P = 128
N = 16384
NCORES = 8
NLOC = N // NCORES          # 2048
NBLK = NLOC // P            # 16
F_IN = 128
DIM = 64
HID = 256
C = 256                     # gathered row width (bf16 -> 512B, dma_gather aligned)
FP = mybir.dt.float32
BF = mybir.dt.bfloat16
I16 = mybir.dt.int16
AF = mybir.ActivationFunctionType
ALU = mybir.AluOpType
AX = mybir.AxisListType
BF_NP = ml_dtypes.bfloat16

LAST_RESULTS = None


def _prep_edges(edge_index):
    src = np.concatenate([edge_index[0], np.arange(N, dtype=np.int64)])
    dst = np.concatenate([edge_index[1], np.arange(N, dtype=np.int64)])
    order = np.argsort(dst, kind="stable")
    src_s, dst_s = src[order], dst[order]
    blk = dst_s // P
    bc = np.bincount(blk, minlength=NCORES * NBLK)
    NT = int(np.ceil(bc.max() / P))
    EB = NT * P
    src_pad = np.zeros((NCORES, NBLK, EB), dtype=np.int64)   # pad idx 0 (valid row)
    dst_pad = np.full((NCORES, NBLK, EB), P, dtype=np.float32)  # P = pad marker
    starts = np.concatenate([[0], np.cumsum(bc)])
    for g in range(NCORES * NBLK):
        c, b = divmod(g, NBLK)
        s, e = starts[g], starts[g + 1]
        k = e - s
        src_pad[c, b, :k] = src_s[s:e]
        dst_pad[c, b, :k] = (dst_s[s:e] - g * P).astype(np.float32)
    # dst one-hot transposed [d, i]: column i = edge i (partition p=i%128, tile t=i//128)
    mt = (dst_pad[:, :, None, :] == np.arange(P, dtype=np.float32)[None, None, :, None])
    mt = mt.astype(BF_NP)                                     # [NC, NBLK, 128, EB]
    # dst_col [NC, NBLK, P, NT]: value of edge i=t*P+p at [p, t]
    dst_col = dst_pad.reshape(NCORES, NBLK, NT, P).transpose(0, 1, 3, 2)
    dst_col = np.ascontiguousarray(dst_col).astype(BF_NP)
    # gather indices wrapped in 16 partitions: w[q, j] = idx[j*16 + q];
    # replicated into partitions 16-31 (tx/rx Q7 cores read their own group)
    idxw = np.zeros((NCORES, NBLK, 128, EB // 16), dtype=np.int16)
    idxw[:, :, :16, :] = src_pad.reshape(NCORES, NBLK, EB // 16, 16).transpose(0, 1, 3, 2)
    idxw[:, :, 16:32, :] = idxw[:, :, :16, :]
    return idxw, dst_col, mt, NT


def _build(NT):
    nc = bacc.Bacc(None)

    def par(name, shape, dtype=BF):
        return nc.declare_dram_parameter(name, list(shape), dtype, isOutput=False)

    xT = par("xT", [F_IN, NLOC])
    idxw = par("idxw", [NBLK, 128, NT * 8], I16)
    dst_col = par("dst_col", [NBLK, P, NT], BF)
    mt_host = par("mt_host", [NBLK, P, NT * P], BF)
    Win = par("Win", [F_IN, DIM]); b_in = par("b_in", [1, DIM])
    Wskip = par("Wskip", [DIM, HID]); bskip = par("bskip", [1, HID])
    WL1 = par("WL1", [DIM, C]); WR1 = par("WR1", [DIM, C])
    WL2 = par("WL2", [HID, C]); WR2 = par("WR2", [HID, C])
    WL3 = par("WL3", [HID, C]); WR3 = par("WR3", [HID, C])
    svec1 = par("svec1", [P, C]); svec2 = par("svec2", [P, C]); svec3 = par("svec3", [P, C])
    Wm1 = par("Wm1", [HID, DIM]); bm1 = par("bm1", [1, DIM])
    Wm2 = par("Wm2", [DIM, DIM]); bm2 = par("bm2", [1, DIM])
    Wm3 = par("Wm3", [DIM, 1]); bm3 = par("bm3", [1, 1])
    iota_f = par("iota_f", [P, P])
    out = nc.declare_dram_parameter("out", [1, NLOC], FP, isOutput=True)

    xl_loc = {l: nc.dram_tensor(f"xl_loc{l}", [NLOC, C], BF) for l in (1, 2, 3)}
    xl_full = {l: nc.dram_tensor(f"xl_full{l}", [N, C], BF, addr_space="Shared")
               for l in (1, 2, 3)}

    with tile.TileContext(nc) as tc:
        with (
            tc.tile_pool(name="const", bufs=1) as cp,
            tc.tile_pool(name="big", bufs=1) as bigp,
            tc.tile_pool(name="wk", bufs=1) as wk,
            tc.tile_pool(name="ps_mm", bufs=2, space="PSUM") as ps_mm,
            tc.tile_pool(name="ps_out", bufs=2, space="PSUM") as ps_out_pool,
            tc.tile_pool(name="ps_w", bufs=2, space="PSUM") as ps_w_pool,
        ):
            def load_const(pname, ap, shape, dtype=BF):
                t = cp.tile(list(shape), dtype, name=pname + "_sb")
                nc.sync.dma_start(out=t[:], in_=ap[:])
                return t

            def load_const_2k(pname, ap, rows, cols):
                assert rows == 2 * P
                t = cp.tile([P, 2 * cols], BF, name=pname + "_sb")
                nc.sync.dma_start(out=t[:, :cols], in_=ap[:P, :])
                nc.sync.dma_start(out=t[:, cols:], in_=ap[P:, :])
                return t

            ident_f = cp.tile([P, P], FP, name="ident_f")
            make_identity(nc, ident_f[:])
            ident_b = cp.tile([P, P], BF, name="ident_b")
            nc.vector.tensor_copy(out=ident_b[:], in_=ident_f[:])
            ones_row = cp.tile([1, 512], BF, name="ones_row")
            nc.vector.memset(ones_row[:], 1.0)

            xT_sb = load_const("xT", xT, [F_IN, NLOC])
            Win_sb = load_const("Win", Win, [F_IN, DIM])
            b_in_sb = load_const("b_in", b_in, [1, DIM])
            Wskip_sb = load_const("Wskip", Wskip, [DIM, HID])
            bskip_sb = load_const("bskip", bskip, [1, HID])
            WL_sb = {1: load_const("WL1", WL1, [DIM, C]),
                     2: load_const_2k("WL2", WL2, HID, C),
                     3: load_const_2k("WL3", WL3, HID, C)}
            WR_sb = {1: load_const("WR1", WR1, [DIM, C]),
                     2: load_const_2k("WR2", WR2, HID, C),
                     3: load_const_2k("WR3", WR3, HID, C)}
            svec_sb = {1: load_const("svec1", svec1, [P, C]),
                       2: load_const("svec2", svec2, [P, C]),
                       3: load_const("svec3", svec3, [P, C])}
            Wm1_sb = load_const_2k("Wm1", Wm1, HID, DIM)
            bm1_sb = load_const("bm1", bm1, [1, DIM])
            Wm2_sb = load_const("Wm2", Wm2, [DIM, DIM])
            bm2_sb = load_const("bm2", bm2, [1, DIM])
            Wm3_sb = load_const("Wm3", Wm3, [DIM, 1])
            bm3_sb = load_const("bm3", bm3, [1, 1])
            iof_sb = load_const("iota_f", iota_f, [P, P])

            actT = {0: bigp.tile([P, NLOC], BF, name="actT0"),
                    1: bigp.tile([P, NLOC], BF, name="actT1")}
            act_prev = bigp.tile([P, NBLK * HID], FP, name="act_prev")
            act_next = bigp.tile([P, NBLK * HID], FP, name="act_next")
            XRb = bigp.tile([P, NBLK * C], BF, name="XRb")
            hT = bigp.tile([DIM, NLOC], BF, name="hT")
            m1T = bigp.tile([DIM, NLOC], BF, name="m1T")
            m2T = bigp.tile([DIM, NLOC], BF, name="m2T")
            y_sb = bigp.tile([1, NLOC], FP, name="y_sb")

            # ---------------- phase A ----------------
            for j in range(NLOC // 512):
                sl = slice(j * 512, (j + 1) * 512)
                pmm = ps_mm.tile([P, 512], FP, space="PSUM", name="pmm", tag="pmm")
                nc.tensor.matmul(out=pmm[:DIM, :], lhsT=Win_sb[:], rhs=xT_sb[:, sl],
                                 start=True, stop=False)
                nc.tensor.matmul(out=pmm[:DIM, :], lhsT=b_in_sb[:], rhs=ones_row[:],
                                 start=False, stop=True)
                nc.scalar.activation(out=hT[:DIM, sl], in_=pmm[:DIM, :], func=AF.Relu)

            for b in range(NBLK):
                nsl = slice(b * P, (b + 1) * P)
                pxl = ps_mm.tile([P, C], FP, space="PSUM", name="pxl", tag="pmm")
                nc.tensor.matmul(out=pxl[:], lhsT=hT[:DIM, nsl], rhs=WL_sb[1][:],
                                 start=True, stop=True)
                xl_st = wk.tile([P, C], BF, name="xl_st", tag="xl_st", bufs=3)
                nc.scalar.activation(out=xl_st[:], in_=pxl[:], func=AF.Copy)
                nc.sync.dma_start(out=xl_loc[1][nsl, :], in_=xl_st[:])

                pxr = ps_mm.tile([P, C], FP, space="PSUM", name="pxr", tag="pmm")
                nc.tensor.matmul(out=pxr[:], lhsT=hT[:DIM, nsl], rhs=WR_sb[1][:],
                                 start=True, stop=True)
                nc.scalar.activation(out=XRb[:, b * C:(b + 1) * C], in_=pxr[:], func=AF.Copy)

                psk = ps_mm.tile([P, HID], FP, space="PSUM", name="psk", tag="pmm")
                nc.tensor.matmul(out=psk[:], lhsT=hT[:DIM, nsl], rhs=Wskip_sb[:],
                                 start=True, stop=False)
                nc.tensor.matmul(out=psk[:], lhsT=ones_row[:, :P], rhs=bskip_sb[:],
                                 start=False, stop=True)
                nc.scalar.activation(out=act_prev[:, b * HID:(b + 1) * HID], in_=psk[:],
                                     func=AF.Copy)

            nc.gpsimd.collective_compute(
                "AllGather", ALU.bypass, replica_groups=[list(range(NCORES))],
                ins=[xl_loc[1][:]], outs=[xl_full[1][:]])

            # ---------------- edge stage ----------------
            def edge_layer(l, H):
                D = C // H
                CV = C + H                      # v row: scaled xl + alpha cols
                for b in range(NBLK):
                    idx_b = wk.tile([128, NT * 8], I16, name="idx_b", tag="idx_b", bufs=2)
                    nc.sync.dma_start(out=idx_b[:], in_=idxw[b])
                    dst_b = wk.tile([P, NT], BF, name="dst_b", tag="dst_b", bufs=2)
                    nc.sync.dma_start(out=dst_b[:], in_=dst_col[b])
                    mt_b = wk.tile([P, NT * P], BF, name="mt_b", tag="mt_b", bufs=2)
                    nc.sync.dma_start(out=mt_b[:], in_=mt_host[b])
                    m_b = wk.tile([P, NT * P], BF, name="m_b", tag="m_b", bufs=2)
                    nc.vector.tensor_tensor(
                        out=m_b[:].rearrange("p (t e) -> p t e", t=NT),
                        in0=dst_b[:, :, None].to_broadcast([P, NT, P]),
                        in1=iof_sb[:, None, :].to_broadcast([P, NT, P]),
                        op=ALU.is_equal)
                    xl_all = wk.tile([P, NT * C], BF, name="xl_all", tag="xl_all", bufs=2)
                    nc.gpsimd.dma_gather(
                        xl_all[:].rearrange("p (t c) -> p t c", t=NT),
                        xl_full[l][:], idx_b[:], NT * P, NT * P, C,
                        single_packet=False)
                    # w = M @ XR + xl  (PE); P = (2/3)|w| on ACT.  Logits:
                    # att.lrelu(w) = 0.6 att.w + 0.4 att.|w|; the xr part of the
                    # linear term is constant per dst so it cancels in the
                    # softmax -> logit ~ sum 0.6 att.(xl + (2/3)|w|).
                    P_all = wk.tile([P, NT * C], BF, name="P_all", tag="P_all", bufs=2)
                    for t in range(NT):
                        w_ps = ps_w_pool.tile([P, C], FP, space="PSUM",
                                              name="w_ps", tag="w_ps")
                        nc.tensor.matmul(out=w_ps[:], lhsT=mt_b[:, t * P:(t + 1) * P],
                                         rhs=XRb[:, b * C:(b + 1) * C],
                                         start=True, stop=False)
                        nc.tensor.matmul(out=w_ps[:], lhsT=ident_b[:],
                                         rhs=xl_all[:, t * C:(t + 1) * C],
                                         start=False, stop=True)
                        nc.scalar.activation(out=P_all[:, t * C:(t + 1) * C],
                                             in_=w_ps[:], func=AF.Abs,
                                             scale=2.0 / 3.0)
                    z_all = wk.tile([P, NT * C], BF, name="z_all", tag="z_all", bufs=2)
                    nc.vector.tensor_tensor(
                        out=z_all[:].rearrange("p (t c) -> p t c", t=NT),
                        in0=P_all[:].rearrange("p (t c) -> p t c", t=NT),
                        in1=xl_all[:].rearrange("p (t c) -> p t c", t=NT),
                        op=ALU.add)
                    nc.vector.tensor_tensor(
                        out=z_all[:].rearrange("p (t c) -> p t c", t=NT),
                        in0=z_all[:].rearrange("p (t c) -> p t c", t=NT),
                        in1=svec_sb[l][:, None, :].to_broadcast([P, NT, C]),
                        op=ALU.mult)
                    r_all = wk.tile([P, NT * H], FP, name="r_all", tag="r_all", bufs=2)
                    nc.vector.tensor_reduce(
                        out=r_all[:].rearrange("p (t h) -> p t h", t=NT),
                        in_=z_all[:].rearrange("p (t h d) -> p t h d", t=NT, h=H),
                        axis=AX.X, op=ALU.add)
                    alpha_all = wk.tile([P, NT * H], FP, name="alpha_all",
                                        tag="alpha_all", bufs=2)
                    nc.scalar.activation(out=alpha_all[:], in_=r_all[:], func=AF.Exp)
                    # v rows: alpha-scaled xl plus alpha columns for the denominator
                    v_all = wk.tile([P, NT * CV], BF, name="v_all", tag="v_all", bufs=2)
                    v_r = v_all[:].rearrange("p (t cv) -> p t cv", t=NT)
                    nc.vector.tensor_tensor(
                        out=v_r[:, :, :C].rearrange("p t (h d) -> p t h d", h=H),
                        in0=xl_all[:].rearrange("p (t h d) -> p t h d", t=NT, h=H),
                        in1=alpha_all[:].rearrange("p (t h) -> p t h", t=NT)[:, :, :, None]
                            .to_broadcast([P, NT, H, D]),
                        op=ALU.mult)
                    nc.vector.tensor_copy(
                        out=v_r[:, :, C:],
                        in_=alpha_all[:].rearrange("p (t h) -> p t h", t=NT))
                    ps_o = ps_out_pool.tile([P, CV], FP, space="PSUM",
                                            name="ps_o", tag="ps_o")
                    for t in range(NT):
                        nc.tensor.matmul(out=ps_o[:], lhsT=m_b[:, t * P:(t + 1) * P],
                                         rhs=v_all[:, t * CV:(t + 1) * CV],
                                         start=(t == 0), stop=(t == NT - 1))
                    # finalize: normalize, relu, residual, transpose
                    rec = wk.tile([P, H], FP, name="rec", tag="rec", bufs=2)
                    nc.vector.reciprocal(out=rec[:], in_=ps_o[:, C:])
                    g_t = wk.tile([P, C], FP, name="g_t", tag="g_t", bufs=2)
                    nc.vector.tensor_tensor(
                        out=g_t[:].rearrange("p (h d) -> p h d", h=H),
                        in0=ps_o[:, :C].rearrange("p (h d) -> p h d", h=H),
                        in1=rec[:, :, None].to_broadcast([P, H, D]),
                        op=ALU.mult)
                    gr = wk.tile([P, C], FP, name="gr", tag="gr", bufs=2)
                    nc.scalar.activation(out=gr[:], in_=g_t[:], func=AF.Relu)
                    bsl = slice(b * HID, (b + 1) * HID)
                    nc.vector.tensor_tensor(out=act_next[:, bsl], in0=gr[:],
                                            in1=act_prev[:, bsl], op=ALU.add)
                    for k in range(2):
                        tp = ps_mm.tile([P, P], FP, space="PSUM", name="tp", tag="pmm")
                        nc.tensor.transpose(
                            out=tp[:], in_=act_next[:, b * HID + k * P:b * HID + (k + 1) * P],
                            identity=ident_f[:])
                        nc.scalar.activation(out=actT[k][:, b * P:(b + 1) * P],
                                             in_=tp[:], func=AF.Copy)

            def xlxr_layer(l):
                for b in range(NBLK):
                    nsl = slice(b * P, (b + 1) * P)
                    pxl = ps_mm.tile([P, C], FP, space="PSUM", name="pxl2", tag="pmm")
                    for k in range(2):
                        nc.tensor.matmul(out=pxl[:], lhsT=actT[k][:, nsl],
                                         rhs=WL_sb[l][:, k * C:(k + 1) * C],
                                         start=(k == 0), stop=(k == 1))
                    xl_st = wk.tile([P, C], BF, name="xl_st2", tag="xl_st", bufs=3)
                    nc.scalar.activation(out=xl_st[:], in_=pxl[:], func=AF.Copy)
                    nc.sync.dma_start(out=xl_loc[l][nsl, :], in_=xl_st[:])

                    pxr = ps_mm.tile([P, C], FP, space="PSUM", name="pxr2", tag="pmm")
                    for k in range(2):
                        nc.tensor.matmul(out=pxr[:], lhsT=actT[k][:, nsl],
                                         rhs=WR_sb[l][:, k * C:(k + 1) * C],
                                         start=(k == 0), stop=(k == 1))
                    nc.scalar.activation(out=XRb[:, b * C:(b + 1) * C], in_=pxr[:],
                                         func=AF.Copy)
                nc.gpsimd.collective_compute(
                    "AllGather", ALU.bypass, replica_groups=[list(range(NCORES))],
                    ins=[xl_loc[l][:]], outs=[xl_full[l][:]])

            edge_layer(1, 4)
            act_prev, act_next = act_next, act_prev
            xlxr_layer(2)
            edge_layer(2, 1)
            act_prev, act_next = act_next, act_prev
            xlxr_layer(3)
            edge_layer(3, 1)

            # ---------------- MLP head ----------------
            for j in range(NLOC // 512):
                sl = slice(j * 512, (j + 1) * 512)
                pm1 = ps_mm.tile([P, 512], FP, space="PSUM", name="pm1", tag="pmm")
                for k in range(2):
                    nc.tensor.matmul(out=pm1[:DIM, :], lhsT=Wm1_sb[:, k * DIM:(k + 1) * DIM],
                                     rhs=actT[k][:, sl], start=(k == 0), stop=False)
                nc.tensor.matmul(out=pm1[:DIM, :], lhsT=bm1_sb[:], rhs=ones_row[:],
                                 start=False, stop=True)
                nc.scalar.activation(out=m1T[:DIM, sl], in_=pm1[:DIM, :], func=AF.Relu)
            for j in range(NLOC // 512):
                sl = slice(j * 512, (j + 1) * 512)
                pm2 = ps_mm.tile([P, 512], FP, space="PSUM", name="pm2", tag="pmm")
                nc.tensor.matmul(out=pm2[:DIM, :], lhsT=Wm2_sb[:], rhs=m1T[:DIM, sl],
                                 start=True, stop=False)
                nc.tensor.matmul(out=pm2[:DIM, :], lhsT=bm2_sb[:], rhs=ones_row[:],
                                 start=False, stop=True)
                nc.scalar.activation(out=m2T[:DIM, sl], in_=pm2[:DIM, :], func=AF.Relu)
            for j in range(NLOC // 512):
                sl = slice(j * 512, (j + 1) * 512)
                py = ps_mm.tile([P, 512], FP, space="PSUM", name="py", tag="pmm")
                nc.tensor.matmul(out=py[:1, :], lhsT=Wm3_sb[:], rhs=m2T[:DIM, sl],
                                 start=True, stop=False)
                nc.tensor.matmul(out=py[:1, :], lhsT=bm3_sb[:], rhs=ones_row[:],
                                 start=False, stop=True)
                nc.scalar.activation(out=y_sb[:, sl], in_=py[:1, :], func=AF.Copy)
            nc.sync.dma_start(out=out[:], in_=y_sb[:])

    nc.compile()
    return nc


_BUILD_CACHE = {}


def _get_program(NT):
    if NT not in _BUILD_CACHE:
        _BUILD_CACHE[NT] = _build(NT)
    return _BUILD_CACHE[NT]


def kernel(**inputs) -> np.ndarray:
    global LAST_RESULTS
    ii = {k: np.asarray(v) for k, v in inputs.items()}
    assert ii["x"].shape == (N, F_IN)
    for l in (1, 2, 3):
        assert not np.any(ii[f"b{l}"]), "GAT bias assumed zero"

    idxw, dst_col, mt, NT = _prep_edges(np.asarray(ii["edge_index"], np.int64))
    iota = np.arange(P, dtype=BF_NP)
    iota_f = np.tile(iota[None, :], (P, 1))

    def bf(a):
        return np.asarray(a, np.float32).astype(BF_NP)

    common = dict(
        Win=bf(ii["Win"]), b_in=bf(ii["b_in"])[None, :],
        Wskip=bf(ii["Wskip"]), bskip=bf(ii["bskip"])[None, :],
        Wm1=bf(ii["Wm1"]), bm1=bf(ii["bm1"])[None, :],
        Wm2=bf(ii["Wm2"]), bm2=bf(ii["bm2"])[None, :],
        Wm3=bf(ii["Wm3"]), bm3=bf(ii["bm3"])[None, :],
        iota_f=iota_f,
    )
    for l in (1, 2, 3):
        common[f"WL{l}"] = bf(ii[f"Wl{l}"])
        common[f"WR{l}"] = bf(ii[f"Wr{l}"])
        attf = np.asarray(ii[f"att{l}"], np.float32).reshape(-1)
        common[f"svec{l}"] = np.tile(0.6 * attf[None, :], (P, 1)).astype(BF_NP)

    x = np.asarray(ii["x"], np.float32)
    in_maps = []
    for c in range(NCORES):
        m = dict(common)
        m["xT"] = np.ascontiguousarray(x[c * NLOC:(c + 1) * NLOC].T).astype(BF_NP)
        m["idxw"] = idxw[c]
        m["dst_col"] = dst_col[c]
        m["mt_host"] = np.ascontiguousarray(mt[c].reshape(NBLK, P, NT * P))
        in_maps.append(m)

    nc = _get_program(NT)
    res = run_bass_kernel_spmd(nc, in_maps, list(range(NCORES)),
                               trace=bool(os.environ.get("GAT_TRACE")))
    LAST_RESULTS = res
    return np.concatenate([res.results[c]["out"].reshape(-1) for c in range(NCORES)])


# revision 7
# speedup vs baseline: 1.0032x; 1.0032x over previous
"""Trainium2 Bass kernel for the 3-layer GATv2 network (nn_GAT_35940286333219).

Sharding: nodes contiguously across 8 cores (2048 each); edges partitioned by
destination so segment-softmax/scatter-add stay local; per-layer AllGather of
the source-side transformed features XL = act @ Wl; per-edge source rows via
one dma_gather per (block, layer).

v3: C=256 (512B rows) so a single dma_gather fetches a whole block's edge rows
(48 SWDGE instructions instead of 864 indirect DMAs); per-edge logits via the
Lrelu activation (alpha=0.2) directly instead of the abs decomposition with
extra weight columns; all per-tile DVE element-wise work batched into
per-block ops with broadcast APs; the transposed one-hot M^T precomputed on
the host and DMA'd instead of per-tile PE transposes + PSUM round trips.
"""
import os
import numpy as np
import ml_dtypes

import concourse.bacc as bacc
import concourse.bass as bass
import concourse.mybir as mybir
import concourse.tile as tile
from concourse.bass_utils import run_bass_kernel_spmd
from concourse.masks import make_identity

P = 128
N = 16384
NCORES = 8
NLOC = N // NCORES          # 2048
NBLK = NLOC // P            # 16
F_IN = 128
DIM = 64
HID = 256
C = 256                     # gathered row width (bf16 -> 512B, dma_gather aligned)
FP = mybir.dt.float32
BF = mybir.dt.bfloat16
I16 = mybir.dt.int16
AF = mybir.ActivationFunctionType
ALU = mybir.AluOpType
AX = mybir.AxisListType
BF_NP = ml_dtypes.bfloat16

NSWQ = 4                    # SWDGE queues used round-robin for the gathers
LAST_RESULTS = None


def _prep_edges(edge_index):
    src = np.concatenate([edge_index[0], np.arange(N, dtype=np.int64)])
    dst = np.concatenate([edge_index[1], np.arange(N, dtype=np.int64)])
    order = np.argsort(dst, kind="stable")
    src_s, dst_s = src[order], dst[order]
    blk = dst_s // P
    bc = np.bincount(blk, minlength=NCORES * NBLK)
    NT = int(np.ceil(bc.max() / P))
    EB = NT * P
    src_pad = np.zeros((NCORES, NBLK, EB), dtype=np.int64)   # pad idx 0 (valid row)
    dst_pad = np.full((NCORES, NBLK, EB), P, dtype=np.float32)  # P = pad marker
    starts = np.concatenate([[0], np.cumsum(bc)])
    for g in range(NCORES * NBLK):
        c, b = divmod(g, NBLK)
        s, e = starts[g], starts[g + 1]
        k = e - s
        src_pad[c, b, :k] = src_s[s:e]
        dst_pad[c, b, :k] = (dst_s[s:e] - g * P).astype(np.float32)
    # dst one-hot transposed [d, i]: column i = edge i (partition p=i%128, tile t=i//128)
    mt = (dst_pad[:, :, None, :] == np.arange(P, dtype=np.float32)[None, None, :, None])
    mt = mt.astype(BF_NP)                                     # [NC, NBLK, 128, EB]
    # dst_col [NC, NBLK, P, NT]: value of edge i=t*P+p at [p, t]
    dst_col = dst_pad.reshape(NCORES, NBLK, NT, P).transpose(0, 1, 3, 2)
    dst_col = np.ascontiguousarray(dst_col).astype(BF_NP)
    # gather indices wrapped in 16 partitions: w[q, j] = idx[j*16 + q];
    # queue q (= block % NQ) reads partitions [q*32, q*32+16) and its tx-core
    # replica [q*32+16, q*32+32)
    idxw = np.zeros((NCORES, NBLK, 128, EB // 16), dtype=np.int16)
    wrapped = src_pad.reshape(NCORES, NBLK, EB // 16, 16).transpose(0, 1, 3, 2)
    for b in range(NBLK):
        q = b % NSWQ
        idxw[:, b, q * 32:q * 32 + 16, :] = wrapped[:, b]
        idxw[:, b, q * 32 + 16:q * 32 + 32, :] = wrapped[:, b]
    return idxw, dst_col, mt, NT


def _build(NT):
    nc = bacc.Bacc(None, num_swdge_queues=NSWQ)

    def par(name, shape, dtype=BF):
        return nc.declare_dram_parameter(name, list(shape), dtype, isOutput=False)

    xT = par("xT", [F_IN, NLOC])
    idxw = par("idxw", [NBLK, 128, NT * 8], I16)
    dst_col = par("dst_col", [NBLK, P, NT], BF)
    mt_host = par("mt_host", [NBLK, P, NT * P], BF)
    Win = par("Win", [F_IN, DIM]); b_in = par("b_in", [1, DIM])
    Wskip = par("Wskip", [DIM, HID]); bskip = par("bskip", [1, HID])
    WL1 = par("WL1", [DIM, C]); WR1 = par("WR1", [DIM, C])
    WL2 = par("WL2", [HID, C]); WR2 = par("WR2", [HID, C])
    WL3 = par("WL3", [HID, C]); WR3 = par("WR3", [HID, C])
    svec1 = par("svec1", [P, C]); svec2 = par("svec2", [P, C]); svec3 = par("svec3", [P, C])
    Wm1 = par("Wm1", [HID, DIM]); bm1 = par("bm1", [1, DIM])
    Wm2 = par("Wm2", [DIM, DIM]); bm2 = par("bm2", [1, DIM])
    Wm3 = par("Wm3", [DIM, 1]); bm3 = par("bm3", [1, 1])
    iota_f = par("iota_f", [P, P])
    out = nc.declare_dram_parameter("out", [1, NLOC], FP, isOutput=True)

    xl_loc = {l: nc.dram_tensor(f"xl_loc{l}", [NLOC, C], BF) for l in (1, 2, 3)}
    xl_full = {l: nc.dram_tensor(f"xl_full{l}", [N, C], BF, addr_space="Shared")
               for l in (1, 2, 3)}

    with tile.TileContext(nc) as tc:
        with (
            tc.tile_pool(name="const", bufs=1) as cp,
            tc.tile_pool(name="big", bufs=1) as bigp,
            tc.tile_pool(name="wk", bufs=1) as wk,
            tc.tile_pool(name="ps_mm", bufs=2, space="PSUM") as ps_mm,
            tc.tile_pool(name="ps_out", bufs=2, space="PSUM") as ps_out_pool,
            tc.tile_pool(name="ps_w", bufs=2, space="PSUM") as ps_w_pool,
        ):
            def load_const(pname, ap, shape, dtype=BF):
                t = cp.tile(list(shape), dtype, name=pname + "_sb")
                nc.sync.dma_start(out=t[:], in_=ap[:])
                return t

            def load_const_2k(pname, ap, rows, cols):
                assert rows == 2 * P
                t = cp.tile([P, 2 * cols], BF, name=pname + "_sb")
                nc.sync.dma_start(out=t[:, :cols], in_=ap[:P, :])
                nc.sync.dma_start(out=t[:, cols:], in_=ap[P:, :])
                return t

            ident_f = cp.tile([P, P], FP, name="ident_f")
            make_identity(nc, ident_f[:])
            ident_b = cp.tile([P, P], BF, name="ident_b")
            nc.vector.tensor_copy(out=ident_b[:], in_=ident_f[:])
            ones_row = cp.tile([1, 512], BF, name="ones_row")
            nc.vector.memset(ones_row[:], 1.0)

            xT_sb = load_const("xT", xT, [F_IN, NLOC])
            Win_sb = load_const("Win", Win, [F_IN, DIM])
            b_in_sb = load_const("b_in", b_in, [1, DIM])
            Wskip_sb = load_const("Wskip", Wskip, [DIM, HID])
            bskip_sb = load_const("bskip", bskip, [1, HID])
            WL_sb = {1: load_const("WL1", WL1, [DIM, C]),
                     2: load_const_2k("WL2", WL2, HID, C),
                     3: load_const_2k("WL3", WL3, HID, C)}
            WR_sb = {1: load_const("WR1", WR1, [DIM, C]),
                     2: load_const_2k("WR2", WR2, HID, C),
                     3: load_const_2k("WR3", WR3, HID, C)}
            svec_sb = {1: load_const("svec1", svec1, [P, C]),
                       2: load_const("svec2", svec2, [P, C]),
                       3: load_const("svec3", svec3, [P, C])}
            Wm1_sb = load_const_2k("Wm1", Wm1, HID, DIM)
            bm1_sb = load_const("bm1", bm1, [1, DIM])
            Wm2_sb = load_const("Wm2", Wm2, [DIM, DIM])
            bm2_sb = load_const("bm2", bm2, [1, DIM])
            Wm3_sb = load_const("Wm3", Wm3, [DIM, 1])
            bm3_sb = load_const("bm3", bm3, [1, 1])
            iof_sb = load_const("iota_f", iota_f, [P, P])

            actT = {0: bigp.tile([P, NLOC], BF, name="actT0"),
                    1: bigp.tile([P, NLOC], BF, name="actT1")}
            act_prev = bigp.tile([P, NBLK * HID], FP, name="act_prev")
            act_next = bigp.tile([P, NBLK * HID], FP, name="act_next")
            XRb = bigp.tile([P, NBLK * C], BF, name="XRb")
            hT = bigp.tile([DIM, NLOC], BF, name="hT")
            m1T = bigp.tile([DIM, NLOC], BF, name="m1T")
            m2T = bigp.tile([DIM, NLOC], BF, name="m2T")
            y_sb = bigp.tile([1, NLOC], FP, name="y_sb")

            # ---------------- phase A ----------------
            for j in range(NLOC // 512):
                sl = slice(j * 512, (j + 1) * 512)
                pmm = ps_mm.tile([P, 512], FP, space="PSUM", name="pmm", tag="pmm")
                nc.tensor.matmul(out=pmm[:DIM, :], lhsT=Win_sb[:], rhs=xT_sb[:, sl],
                                 start=True, stop=False)
                nc.tensor.matmul(out=pmm[:DIM, :], lhsT=b_in_sb[:], rhs=ones_row[:],
                                 start=False, stop=True)
                nc.scalar.activation(out=hT[:DIM, sl], in_=pmm[:DIM, :], func=AF.Relu)

            for b in range(NBLK):
                nsl = slice(b * P, (b + 1) * P)
                pxl = ps_mm.tile([P, C], FP, space="PSUM", name="pxl", tag="pmm")
                nc.tensor.matmul(out=pxl[:], lhsT=hT[:DIM, nsl], rhs=WL_sb[1][:],
                                 start=True, stop=True)
                xl_st = wk.tile([P, C], BF, name="xl_st", tag="xl_st", bufs=3)
                nc.scalar.activation(out=xl_st[:], in_=pxl[:], func=AF.Copy)
                nc.sync.dma_start(out=xl_loc[1][nsl, :], in_=xl_st[:])

                pxr = ps_mm.tile([P, C], FP, space="PSUM", name="pxr", tag="pmm")
                nc.tensor.matmul(out=pxr[:], lhsT=hT[:DIM, nsl], rhs=WR_sb[1][:],
                                 start=True, stop=True)
                nc.scalar.activation(out=XRb[:, b * C:(b + 1) * C], in_=pxr[:], func=AF.Copy)

                psk = ps_mm.tile([P, HID], FP, space="PSUM", name="psk", tag="pmm")
                nc.tensor.matmul(out=psk[:], lhsT=hT[:DIM, nsl], rhs=Wskip_sb[:],
                                 start=True, stop=False)
                nc.tensor.matmul(out=psk[:], lhsT=ones_row[:, :P], rhs=bskip_sb[:],
                                 start=False, stop=True)
                nc.scalar.activation(out=act_prev[:, b * HID:(b + 1) * HID], in_=psk[:],
                                     func=AF.Copy)

            nc.gpsimd.collective_compute(
                "AllGather", ALU.bypass, replica_groups=[list(range(NCORES))],
                ins=[xl_loc[1][:]], outs=[xl_full[1][:]])

            # ---------------- edge stage ----------------
            def edge_layer(l, H):
                D = C // H
                CV = C + H                      # v row: scaled xl + alpha cols
                for b in range(NBLK):
                    idx_b = wk.tile([128, NT * 8], I16, name="idx_b", tag="idx_b", bufs=2)
                    nc.sync.dma_start(out=idx_b[:], in_=idxw[b])
                    dst_b = wk.tile([P, NT], BF, name="dst_b", tag="dst_b", bufs=2)
                    nc.sync.dma_start(out=dst_b[:], in_=dst_col[b])
                    mt_b = wk.tile([P, NT * P], BF, name="mt_b", tag="mt_b", bufs=2)
                    nc.sync.dma_start(out=mt_b[:], in_=mt_host[b])
                    m_b = wk.tile([P, NT * P], BF, name="m_b", tag="m_b", bufs=2)
                    nc.vector.tensor_tensor(
                        out=m_b[:].rearrange("p (t e) -> p t e", t=NT),
                        in0=dst_b[:, :, None].to_broadcast([P, NT, P]),
                        in1=iof_sb[:, None, :].to_broadcast([P, NT, P]),
                        op=ALU.is_equal)
                    xl_all = wk.tile([P, NT * C], BF, name="xl_all", tag="xl_all", bufs=3)
                    nc.gpsimd.dma_gather(
                        xl_all[:].rearrange("p (t c) -> p t c", t=NT),
                        xl_full[l][:], idx_b[:], NT * P, NT * P, C,
                        single_packet=False, queue_num=b % NSWQ)
                    # w = M @ XR + xl  (PE); P = (2/3)|w| on ACT.  Logits:
                    # att.lrelu(w) = 0.6 att.w + 0.4 att.|w|; the xr part of the
                    # linear term is constant per dst so it cancels in the
                    # softmax -> logit ~ sum 0.6 att.(xl + (2/3)|w|).
                    P_all = wk.tile([P, NT * C], BF, name="P_all", tag="P_all", bufs=2)
                    for t in range(NT):
                        w_ps = ps_w_pool.tile([P, C], FP, space="PSUM",
                                              name="w_ps", tag="w_ps")
                        nc.tensor.matmul(out=w_ps[:], lhsT=mt_b[:, t * P:(t + 1) * P],
                                         rhs=XRb[:, b * C:(b + 1) * C],
                                         start=True, stop=False)
                        nc.tensor.matmul(out=w_ps[:], lhsT=ident_b[:],
                                         rhs=xl_all[:, t * C:(t + 1) * C],
                                         start=False, stop=True)
                        nc.scalar.activation(out=P_all[:, t * C:(t + 1) * C],
                                             in_=w_ps[:], func=AF.Abs,
                                             scale=2.0 / 3.0)
                    z_all = wk.tile([P, NT * C], BF, name="z_all", tag="z_all", bufs=2)
                    nc.vector.tensor_tensor(
                        out=z_all[:].rearrange("p (t c) -> p t c", t=NT),
                        in0=P_all[:].rearrange("p (t c) -> p t c", t=NT),
                        in1=xl_all[:].rearrange("p (t c) -> p t c", t=NT),
                        op=ALU.add)
                    nc.vector.tensor_tensor(
                        out=z_all[:].rearrange("p (t c) -> p t c", t=NT),
                        in0=z_all[:].rearrange("p (t c) -> p t c", t=NT),
                        in1=svec_sb[l][:, None, :].to_broadcast([P, NT, C]),
                        op=ALU.mult)
                    r_all = wk.tile([P, NT * H], FP, name="r_all", tag="r_all", bufs=2)
                    nc.vector.tensor_reduce(
                        out=r_all[:].rearrange("p (t h) -> p t h", t=NT),
                        in_=z_all[:].rearrange("p (t h d) -> p t h d", t=NT, h=H),
                        axis=AX.X, op=ALU.add)
                    alpha_all = wk.tile([P, NT * H], FP, name="alpha_all",
                                        tag="alpha_all", bufs=2)
                    nc.scalar.activation(out=alpha_all[:], in_=r_all[:], func=AF.Exp)
                    # v rows: alpha-scaled xl plus alpha columns for the denominator
                    v_all = wk.tile([P, NT * CV], BF, name="v_all", tag="v_all", bufs=2)
                    v_r = v_all[:].rearrange("p (t cv) -> p t cv", t=NT)
                    nc.vector.tensor_tensor(
                        out=v_r[:, :, :C].rearrange("p t (h d) -> p t h d", h=H),
                        in0=xl_all[:].rearrange("p (t h d) -> p t h d", t=NT, h=H),
                        in1=alpha_all[:].rearrange("p (t h) -> p t h", t=NT)[:, :, :, None]
                            .to_broadcast([P, NT, H, D]),
                        op=ALU.mult)
                    nc.vector.tensor_copy(
                        out=v_r[:, :, C:],
                        in_=alpha_all[:].rearrange("p (t h) -> p t h", t=NT))
                    ps_o = ps_out_pool.tile([P, CV], FP, space="PSUM",
                                            name="ps_o", tag="ps_o")
                    for t in range(NT):
                        nc.tensor.matmul(out=ps_o[:], lhsT=m_b[:, t * P:(t + 1) * P],
                                         rhs=v_all[:, t * CV:(t + 1) * CV],
                                         start=(t == 0), stop=(t == NT - 1))
                    # finalize: normalize, relu, residual, transpose
                    rec = wk.tile([P, H], FP, name="rec", tag="rec", bufs=2)
                    nc.vector.reciprocal(out=rec[:], in_=ps_o[:, C:])
                    g_t = wk.tile([P, C], FP, name="g_t", tag="g_t", bufs=2)
                    nc.vector.tensor_tensor(
                        out=g_t[:].rearrange("p (h d) -> p h d", h=H),
                        in0=ps_o[:, :C].rearrange("p (h d) -> p h d", h=H),
                        in1=rec[:, :, None].to_broadcast([P, H, D]),
                        op=ALU.mult)
                    gr = wk.tile([P, C], FP, name="gr", tag="gr", bufs=2)
                    nc.scalar.activation(out=gr[:], in_=g_t[:], func=AF.Relu)
                    bsl = slice(b * HID, (b + 1) * HID)
                    nc.vector.tensor_tensor(out=act_next[:, bsl], in0=gr[:],
                                            in1=act_prev[:, bsl], op=ALU.add)
                    for k in range(2):
                        tp = ps_mm.tile([P, P], FP, space="PSUM", name="tp", tag="pmm")
                        nc.tensor.transpose(
                            out=tp[:], in_=act_next[:, b * HID + k * P:b * HID + (k + 1) * P],
                            identity=ident_f[:])
                        nc.scalar.activation(out=actT[k][:, b * P:(b + 1) * P],
                                             in_=tp[:], func=AF.Copy)

            def xlxr_layer(l):
                for b in range(NBLK):
                    nsl = slice(b * P, (b + 1) * P)
                    pxl = ps_mm.tile([P, C], FP, space="PSUM", name="pxl2", tag="pmm")
                    for k in range(2):
                        nc.tensor.matmul(out=pxl[:], lhsT=actT[k][:, nsl],
                                         rhs=WL_sb[l][:, k * C:(k + 1) * C],
                                         start=(k == 0), stop=(k == 1))
                    xl_st = wk.tile([P, C], BF, name="xl_st2", tag="xl_st", bufs=3)
                    nc.scalar.activation(out=xl_st[:], in_=pxl[:], func=AF.Copy)
                    nc.sync.dma_start(out=xl_loc[l][nsl, :], in_=xl_st[:])

                    pxr = ps_mm.tile([P, C], FP, space="PSUM", name="pxr2", tag="pmm")
                    for k in range(2):
                        nc.tensor.matmul(out=pxr[:], lhsT=actT[k][:, nsl],
                                         rhs=WR_sb[l][:, k * C:(k + 1) * C],
                                         start=(k == 0), stop=(k == 1))
                    nc.scalar.activation(out=XRb[:, b * C:(b + 1) * C], in_=pxr[:],
                                         func=AF.Copy)
                nc.gpsimd.collective_compute(
                    "AllGather", ALU.bypass, replica_groups=[list(range(NCORES))],
                    ins=[xl_loc[l][:]], outs=[xl_full[l][:]])

            edge_layer(1, 4)
            act_prev, act_next = act_next, act_prev
            xlxr_layer(2)
            edge_layer(2, 1)
            act_prev, act_next = act_next, act_prev
            xlxr_layer(3)
            edge_layer(3, 1)

            # ---------------- MLP head ----------------
            for j in range(NLOC // 512):
                sl = slice(j * 512, (j + 1) * 512)
                pm1 = ps_mm.tile([P, 512], FP, space="PSUM", name="pm1", tag="pmm")
                for k in range(2):
                    nc.tensor.matmul(out=pm1[:DIM, :], lhsT=Wm1_sb[:, k * DIM:(k + 1) * DIM],
                                     rhs=actT[k][:, sl], start=(k == 0), stop=False)
                nc.tensor.matmul(out=pm1[:DIM, :], lhsT=bm1_sb[:], rhs=ones_row[:],
                                 start=False, stop=True)
                nc.scalar.activation(out=m1T[:DIM, sl], in_=pm1[:DIM, :], func=AF.Relu)
            for j in range(NLOC // 512):
                sl = slice(j * 512, (j + 1) * 512)
                pm2 = ps_mm.tile([P, 512], FP, space="PSUM", name="pm2", tag="pmm")
                nc.tensor.matmul(out=pm2[:DIM, :], lhsT=Wm2_sb[:], rhs=m1T[:DIM, sl],
                                 start=True, stop=False)
                nc.tensor.matmul(out=pm2[:DIM, :], lhsT=bm2_sb[:], rhs=ones_row[:],
                                 start=False, stop=True)
                nc.scalar.activation(out=m2T[:DIM, sl], in_=pm2[:DIM, :], func=AF.Relu)
            for j in range(NLOC // 512):
                sl = slice(j * 512, (j + 1) * 512)
                py = ps_mm.tile([P, 512], FP, space="PSUM", name="py", tag="pmm")
                nc.tensor.matmul(out=py[:1, :], lhsT=Wm3_sb[:], rhs=m2T[:DIM, sl],
                                 start=True, stop=False)
                nc.tensor.matmul(out=py[:1, :], lhsT=bm3_sb[:], rhs=ones_row[:],
                                 start=False, stop=True)
                nc.scalar.activation(out=y_sb[:, sl], in_=py[:1, :], func=AF.Copy)
            nc.sync.dma_start(out=out[:], in_=y_sb[:])

    nc.compile()
    return nc


_BUILD_CACHE = {}


def _get_program(NT):
    if NT not in _BUILD_CACHE:
        _BUILD_CACHE[NT] = _build(NT)
    return _BUILD_CACHE[NT]


def kernel(**inputs) -> np.ndarray:
    global LAST_RESULTS
    ii = {k: np.asarray(v) for k, v in inputs.items()}
    assert ii["x"].shape == (N, F_IN)
    for l in (1, 2, 3):
        assert not np.any(ii[f"b{l}"]), "GAT bias assumed zero"

    idxw, dst_col, mt, NT = _prep_edges(np.asarray(ii["edge_index"], np.int64))
    iota = np.arange(P, dtype=BF_NP)
    iota_f = np.tile(iota[None, :], (P, 1))

    def bf(a):
        return np.asarray(a, np.float32).astype(BF_NP)

    common = dict(
        Win=bf(ii["Win"]), b_in=bf(ii["b_in"])[None, :],
        Wskip=bf(ii["Wskip"]), bskip=bf(ii["bskip"])[None, :],
        Wm1=bf(ii["Wm1"]), bm1=bf(ii["bm1"])[None, :],
        Wm2=bf(ii["Wm2"]), bm2=bf(ii["bm2"])[None, :],
        Wm3=bf(ii["Wm3"]), bm3=bf(ii["bm3"])[None, :],
        iota_f=iota_f,
    )
    for l in (1, 2, 3):
        common[f"WL{l}"] = bf(ii[f"Wl{l}"])
        common[f"WR{l}"] = bf(ii[f"Wr{l}"])
        attf = np.asarray(ii[f"att{l}"], np.float32).reshape(-1)
        common[f"svec{l}"] = np.tile(0.6 * attf[None, :], (P, 1)).astype(BF_NP)

    x = np.asarray(ii["x"], np.float32)
    in_maps = []
    for c in range(NCORES):
        m = dict(common)
        m["xT"] = np.ascontiguousarray(x[c * NLOC:(c + 1) * NLOC].T).astype(BF_NP)
        m["idxw"] = idxw[c]
        m["dst_col"] = dst_col[c]
        m["mt_host"] = np.ascontiguousarray(mt[c].reshape(NBLK, P, NT * P))
        in_maps.append(m)

    nc = _get_program(NT)
    res = run_bass_kernel_spmd(nc, in_maps, list(range(NCORES)),
                               trace=bool(os.environ.get("GAT_TRACE")))
    LAST_RESULTS = res
    return np.concatenate([res.results[c]["out"].reshape(-1) for c in range(NCORES)])


# revision 10
# speedup vs baseline: 1.1334x; 1.1298x over previous
"""Trainium2 Bass kernel for the 3-layer GATv2 network (nn_GAT_35940286333219).

Sharding: nodes contiguously across 8 cores (2048 each); edges partitioned by
destination so segment-softmax/scatter-add stay local; per-layer AllGather of
the source-side transformed features XL = act @ Wl; per-edge source rows via
one dma_gather per (block, layer).

v3: C=256 (512B rows) so a single dma_gather fetches a whole block's edge rows
(48 SWDGE instructions instead of 864 indirect DMAs); per-edge logits via the
Lrelu activation (alpha=0.2) directly instead of the abs decomposition with
extra weight columns; all per-tile DVE element-wise work batched into
per-block ops with broadcast APs; the transposed one-hot M^T precomputed on
the host and DMA'd instead of per-tile PE transposes + PSUM round trips.
"""
import os
import numpy as np
import ml_dtypes

import concourse.bacc as bacc
import concourse.bass as bass
import concourse.mybir as mybir
import concourse.tile as tile
from concourse.bass_utils import run_bass_kernel_spmd
from concourse.masks import make_identity

P = 128
N = 16384
NCORES = 8
NLOC = N // NCORES          # 2048
NBLK = NLOC // P            # 16
F_IN = 128
DIM = 64
HID = 256
C = 256                     # gathered row width (bf16 -> 512B, dma_gather aligned)
FP = mybir.dt.float32
BF = mybir.dt.bfloat16
I16 = mybir.dt.int16
AF = mybir.ActivationFunctionType
ALU = mybir.AluOpType
AX = mybir.AxisListType
BF_NP = ml_dtypes.bfloat16

LAST_RESULTS = None


def _prep_edges(edge_index):
    src = np.concatenate([edge_index[0], np.arange(N, dtype=np.int64)])
    dst = np.concatenate([edge_index[1], np.arange(N, dtype=np.int64)])
    order = np.argsort(dst, kind="stable")
    src_s, dst_s = src[order], dst[order]
    blk = dst_s // P
    bc = np.bincount(blk, minlength=NCORES * NBLK)
    NT = int(np.ceil(bc.max() / P))
    EB = NT * P
    src_pad = np.zeros((NCORES, NBLK, EB), dtype=np.int64)   # pad idx 0 (valid row)
    dst_pad = np.full((NCORES, NBLK, EB), P, dtype=np.float32)  # P = pad marker
    starts = np.concatenate([[0], np.cumsum(bc)])
    for g in range(NCORES * NBLK):
        c, b = divmod(g, NBLK)
        s, e = starts[g], starts[g + 1]
        k = e - s
        src_pad[c, b, :k] = src_s[s:e]
        dst_pad[c, b, :k] = (dst_s[s:e] - g * P).astype(np.float32)
    # dst one-hot transposed [d, i]: column i = edge i (partition p=i%128, tile t=i//128)
    mt = (dst_pad[:, :, None, :] == np.arange(P, dtype=np.float32)[None, None, :, None])
    mt = mt.astype(BF_NP)                                     # [NC, NBLK, 128, EB]
    # dst_col [NC, NBLK, P, NT]: value of edge i=t*P+p at [p, t]
    dst_col = dst_pad.reshape(NCORES, NBLK, NT, P).transpose(0, 1, 3, 2)
    dst_col = np.ascontiguousarray(dst_col).astype(BF_NP)
    # gather offsets [p, t]: row index of edge i = t*P + p
    src_col = src_pad.reshape(NCORES, NBLK, NT, P).transpose(0, 1, 3, 2)
    src_col = np.ascontiguousarray(src_col).astype(np.int32)
    return src_col, dst_col, mt, NT


def _build(NT):
    nc = bacc.Bacc(None)

    def par(name, shape, dtype=BF):
        return nc.declare_dram_parameter(name, list(shape), dtype, isOutput=False)

    xT = par("xT", [F_IN, NLOC])
    src_col = par("src_col", [NBLK, P, NT], mybir.dt.int32)
    dst_col = par("dst_col", [NBLK, P, NT], BF)
    mt_host = par("mt_host", [NBLK, P, NT * P], BF)
    Win = par("Win", [F_IN, DIM]); b_in = par("b_in", [1, DIM])
    Wskip = par("Wskip", [DIM, HID]); bskip = par("bskip", [1, HID])
    WL1 = par("WL1", [DIM, C]); WR1 = par("WR1", [DIM, C])
    WL2 = par("WL2", [HID, C]); WR2 = par("WR2", [HID, C])
    WL3 = par("WL3", [HID, C]); WR3 = par("WR3", [HID, C])
    svec1 = par("svec1", [P, C]); svec2 = par("svec2", [P, C]); svec3 = par("svec3", [P, C])
    Wm1 = par("Wm1", [HID, DIM]); bm1 = par("bm1", [1, DIM])
    Wm2 = par("Wm2", [DIM, DIM]); bm2 = par("bm2", [1, DIM])
    Wm3 = par("Wm3", [DIM, 1]); bm3 = par("bm3", [1, 1])
    iota_f = par("iota_f", [P, P])
    out = nc.declare_dram_parameter("out", [1, NLOC], FP, isOutput=True)

    xl_loc = {l: nc.dram_tensor(f"xl_loc{l}", [NLOC, C], BF) for l in (1, 2, 3)}
    xl_full = {l: nc.dram_tensor(f"xl_full{l}", [N, C], BF, addr_space="Shared")
               for l in (1, 2, 3)}

    with tile.TileContext(nc) as tc:
        with (
            tc.tile_pool(name="const", bufs=1) as cp,
            tc.tile_pool(name="big", bufs=1) as bigp,
            tc.tile_pool(name="wk", bufs=1) as wk,
            tc.tile_pool(name="ps_mm", bufs=2, space="PSUM") as ps_mm,
            tc.tile_pool(name="ps_out", bufs=2, space="PSUM") as ps_out_pool,
            tc.tile_pool(name="ps_w", bufs=2, space="PSUM") as ps_w_pool,
        ):
            def load_const(pname, ap, shape, dtype=BF):
                t = cp.tile(list(shape), dtype, name=pname + "_sb")
                nc.sync.dma_start(out=t[:], in_=ap[:])
                return t

            def load_const_2k(pname, ap, rows, cols):
                assert rows == 2 * P
                t = cp.tile([P, 2 * cols], BF, name=pname + "_sb")
                nc.sync.dma_start(out=t[:, :cols], in_=ap[:P, :])
                nc.sync.dma_start(out=t[:, cols:], in_=ap[P:, :])
                return t

            ident_f = cp.tile([P, P], FP, name="ident_f")
            make_identity(nc, ident_f[:])
            ident_b = cp.tile([P, P], BF, name="ident_b")
            nc.vector.tensor_copy(out=ident_b[:], in_=ident_f[:])
            ones_row = cp.tile([1, 512], BF, name="ones_row")
            nc.vector.memset(ones_row[:], 1.0)

            xT_sb = load_const("xT", xT, [F_IN, NLOC])
            Win_sb = load_const("Win", Win, [F_IN, DIM])
            b_in_sb = load_const("b_in", b_in, [1, DIM])
            Wskip_sb = load_const("Wskip", Wskip, [DIM, HID])
            bskip_sb = load_const("bskip", bskip, [1, HID])
            WL_sb = {1: load_const("WL1", WL1, [DIM, C]),
                     2: load_const_2k("WL2", WL2, HID, C),
                     3: load_const_2k("WL3", WL3, HID, C)}
            WR_sb = {1: load_const("WR1", WR1, [DIM, C]),
                     2: load_const_2k("WR2", WR2, HID, C),
                     3: load_const_2k("WR3", WR3, HID, C)}
            svec_sb = {1: load_const("svec1", svec1, [P, C]),
                       2: load_const("svec2", svec2, [P, C]),
                       3: load_const("svec3", svec3, [P, C])}
            Wm1_sb = load_const_2k("Wm1", Wm1, HID, DIM)
            bm1_sb = load_const("bm1", bm1, [1, DIM])
            Wm2_sb = load_const("Wm2", Wm2, [DIM, DIM])
            bm2_sb = load_const("bm2", bm2, [1, DIM])
            Wm3_sb = load_const("Wm3", Wm3, [DIM, 1])
            bm3_sb = load_const("bm3", bm3, [1, 1])
            iof_sb = load_const("iota_f", iota_f, [P, P])

            actT = {0: bigp.tile([P, NLOC], BF, name="actT0"),
                    1: bigp.tile([P, NLOC], BF, name="actT1")}
            act_prev = bigp.tile([P, NBLK * HID], FP, name="act_prev")
            act_next = bigp.tile([P, NBLK * HID], FP, name="act_next")
            XRb = bigp.tile([P, NBLK * C], BF, name="XRb")
            hT = bigp.tile([DIM, NLOC], BF, name="hT")
            m1T = bigp.tile([DIM, NLOC], BF, name="m1T")
            m2T = bigp.tile([DIM, NLOC], BF, name="m2T")
            y_sb = bigp.tile([1, NLOC], FP, name="y_sb")

            # ---------------- phase A ----------------
            for j in range(NLOC // 512):
                sl = slice(j * 512, (j + 1) * 512)
                pmm = ps_mm.tile([P, 512], FP, space="PSUM", name="pmm", tag="pmm")
                nc.tensor.matmul(out=pmm[:DIM, :], lhsT=Win_sb[:], rhs=xT_sb[:, sl],
                                 start=True, stop=False)
                nc.tensor.matmul(out=pmm[:DIM, :], lhsT=b_in_sb[:], rhs=ones_row[:],
                                 start=False, stop=True)
                nc.scalar.activation(out=hT[:DIM, sl], in_=pmm[:DIM, :], func=AF.Relu)

            for b in range(NBLK):
                nsl = slice(b * P, (b + 1) * P)
                pxl = ps_mm.tile([P, C], FP, space="PSUM", name="pxl", tag="pmm")
                nc.tensor.matmul(out=pxl[:], lhsT=hT[:DIM, nsl], rhs=WL_sb[1][:],
                                 start=True, stop=True)
                xl_st = wk.tile([P, C], BF, name="xl_st", tag="xl_st", bufs=3)
                nc.scalar.activation(out=xl_st[:], in_=pxl[:], func=AF.Copy)
                nc.sync.dma_start(out=xl_loc[1][nsl, :], in_=xl_st[:])

                pxr = ps_mm.tile([P, C], FP, space="PSUM", name="pxr", tag="pmm")
                nc.tensor.matmul(out=pxr[:], lhsT=hT[:DIM, nsl], rhs=WR_sb[1][:],
                                 start=True, stop=True)
                nc.scalar.activation(out=XRb[:, b * C:(b + 1) * C], in_=pxr[:], func=AF.Copy)

                psk = ps_mm.tile([P, HID], FP, space="PSUM", name="psk", tag="pmm")
                nc.tensor.matmul(out=psk[:], lhsT=hT[:DIM, nsl], rhs=Wskip_sb[:],
                                 start=True, stop=False)
                nc.tensor.matmul(out=psk[:], lhsT=ones_row[:, :P], rhs=bskip_sb[:],
                                 start=False, stop=True)
                nc.scalar.activation(out=act_prev[:, b * HID:(b + 1) * HID], in_=psk[:],
                                     func=AF.Copy)

            nc.gpsimd.collective_compute(
                "AllGather", ALU.bypass, replica_groups=[list(range(NCORES))],
                ins=[xl_loc[1][:]], outs=[xl_full[1][:]])

            # ---------------- edge stage ----------------
            svec_rep = bigp.tile([P, NT * C], BF, name="svec_rep")

            def edge_layer(l, H):
                D = C // H
                CV = C + H                      # v row: scaled xl + alpha cols
                nc.vector.tensor_copy(
                    out=svec_rep[:].rearrange("p (t c) -> p t c", t=NT),
                    in_=svec_sb[l][:, None, :].to_broadcast([P, NT, C]))
                for b in range(NBLK):
                    src_b = wk.tile([P, NT], mybir.dt.int32, name="src_b",
                                    tag="src_b", bufs=2)
                    nc.sync.dma_start(out=src_b[:], in_=src_col[b])
                    dst_b = wk.tile([P, NT], BF, name="dst_b", tag="dst_b", bufs=2)
                    nc.sync.dma_start(out=dst_b[:], in_=dst_col[b])
                    mt_b = wk.tile([P, NT * P], BF, name="mt_b", tag="mt_b", bufs=2)
                    nc.sync.dma_start(out=mt_b[:], in_=mt_host[b])
                    m_b = wk.tile([P, NT * P], BF, name="m_b", tag="m_b", bufs=2)
                    nc.vector.tensor_tensor(
                        out=m_b[:].rearrange("p (t e) -> p t e", t=NT),
                        in0=dst_b[:, :, None].to_broadcast([P, NT, P]),
                        in1=iof_sb[:, None, :].to_broadcast([P, NT, P]),
                        op=ALU.is_equal)
                    xl_all = wk.tile([P, NT * C], BF, name="xl_all", tag="xl_all", bufs=3)
                    for t in range(NT):
                        nc.gpsimd.indirect_dma_start(
                            out=xl_all[:, t * C:(t + 1) * C], out_offset=None,
                            in_=xl_full[l][:],
                            in_offset=bass.IndirectOffsetOnAxis(
                                ap=src_b[:, t:t + 1], axis=0))
                    # w = M @ XR + xl  (PE); P = (2/3)|w| on ACT.  Logits:
                    # att.lrelu(w) = 0.6 att.w + 0.4 att.|w|; the xr part of the
                    # linear term is constant per dst so it cancels in the
                    # softmax -> logit ~ sum 0.6 att.(xl + (2/3)|w|).
                    P_all = wk.tile([P, NT * C], BF, name="P_all", tag="P_all", bufs=2)
                    for t in range(NT):
                        w_ps = ps_w_pool.tile([P, C], FP, space="PSUM",
                                              name="w_ps", tag="w_ps")
                        nc.tensor.matmul(out=w_ps[:], lhsT=mt_b[:, t * P:(t + 1) * P],
                                         rhs=XRb[:, b * C:(b + 1) * C],
                                         start=True, stop=False)
                        nc.tensor.matmul(out=w_ps[:], lhsT=ident_b[:],
                                         rhs=xl_all[:, t * C:(t + 1) * C],
                                         start=False, stop=True)
                        nc.scalar.activation(out=P_all[:, t * C:(t + 1) * C],
                                             in_=w_ps[:], func=AF.Abs,
                                             scale=2.0 / 3.0)
                    z_all = wk.tile([P, NT * C], BF, name="z_all", tag="z_all", bufs=2)
                    nc.vector.tensor_tensor(out=z_all[:], in0=P_all[:],
                                            in1=xl_all[:], op=ALU.add)
                    nc.vector.tensor_tensor(out=z_all[:], in0=z_all[:],
                                            in1=svec_rep[:], op=ALU.mult)
                    r_all = wk.tile([P, NT * H], FP, name="r_all", tag="r_all", bufs=2)
                    nc.vector.tensor_reduce(
                        out=r_all[:].rearrange("p (t h) -> p t h", t=NT),
                        in_=z_all[:].rearrange("p (t h d) -> p t h d", t=NT, h=H),
                        axis=AX.X, op=ALU.add)
                    alpha_all = wk.tile([P, NT * H], BF, name="alpha_all",
                                        tag="alpha_all", bufs=2)
                    nc.scalar.activation(out=alpha_all[:], in_=r_all[:], func=AF.Exp)
                    # v rows: alpha-scaled xl plus alpha columns for the denominator
                    v_all = wk.tile([P, NT * CV], BF, name="v_all", tag="v_all", bufs=2)
                    v_r = v_all[:].rearrange("p (t cv) -> p t cv", t=NT)
                    nc.vector.tensor_tensor(
                        out=v_r[:, :, :C].rearrange("p t (h d) -> p t h d", h=H),
                        in0=xl_all[:].rearrange("p (t h d) -> p t h d", t=NT, h=H),
                        in1=alpha_all[:].rearrange("p (t h) -> p t h", t=NT)[:, :, :, None]
                            .to_broadcast([P, NT, H, D]),
                        op=ALU.mult)
                    nc.vector.tensor_copy(
                        out=v_r[:, :, C:],
                        in_=alpha_all[:].rearrange("p (t h) -> p t h", t=NT))
                    ps_o = ps_out_pool.tile([P, CV], FP, space="PSUM",
                                            name="ps_o", tag="ps_o")
                    for t in range(NT):
                        nc.tensor.matmul(out=ps_o[:], lhsT=m_b[:, t * P:(t + 1) * P],
                                         rhs=v_all[:, t * CV:(t + 1) * CV],
                                         start=(t == 0), stop=(t == NT - 1))
                    # finalize: normalize, relu, residual, transpose
                    rec = wk.tile([P, H], FP, name="rec", tag="rec", bufs=2)
                    nc.vector.reciprocal(out=rec[:], in_=ps_o[:, C:])
                    g_t = wk.tile([P, C], FP, name="g_t", tag="g_t", bufs=2)
                    nc.vector.tensor_tensor(
                        out=g_t[:].rearrange("p (h d) -> p h d", h=H),
                        in0=ps_o[:, :C].rearrange("p (h d) -> p h d", h=H),
                        in1=rec[:, :, None].to_broadcast([P, H, D]),
                        op=ALU.mult)
                    gr = wk.tile([P, C], FP, name="gr", tag="gr", bufs=2)
                    nc.scalar.activation(out=gr[:], in_=g_t[:], func=AF.Relu)
                    bsl = slice(b * HID, (b + 1) * HID)
                    nc.vector.tensor_tensor(out=act_next[:, bsl], in0=gr[:],
                                            in1=act_prev[:, bsl], op=ALU.add)
                    for k in range(2):
                        tp = ps_mm.tile([P, P], FP, space="PSUM", name="tp", tag="pmm")
                        nc.tensor.transpose(
                            out=tp[:], in_=act_next[:, b * HID + k * P:b * HID + (k + 1) * P],
                            identity=ident_f[:])
                        nc.scalar.activation(out=actT[k][:, b * P:(b + 1) * P],
                                             in_=tp[:], func=AF.Copy)

            def xlxr_layer(l):
                for b in range(NBLK):
                    nsl = slice(b * P, (b + 1) * P)
                    pxl = ps_mm.tile([P, C], FP, space="PSUM", name="pxl2", tag="pmm")
                    for k in range(2):
                        nc.tensor.matmul(out=pxl[:], lhsT=actT[k][:, nsl],
                                         rhs=WL_sb[l][:, k * C:(k + 1) * C],
                                         start=(k == 0), stop=(k == 1))
                    xl_st = wk.tile([P, C], BF, name="xl_st2", tag="xl_st", bufs=3)
                    nc.scalar.activation(out=xl_st[:], in_=pxl[:], func=AF.Copy)
                    nc.sync.dma_start(out=xl_loc[l][nsl, :], in_=xl_st[:])

                    pxr = ps_mm.tile([P, C], FP, space="PSUM", name="pxr2", tag="pmm")
                    for k in range(2):
                        nc.tensor.matmul(out=pxr[:], lhsT=actT[k][:, nsl],
                                         rhs=WR_sb[l][:, k * C:(k + 1) * C],
                                         start=(k == 0), stop=(k == 1))
                    nc.scalar.activation(out=XRb[:, b * C:(b + 1) * C], in_=pxr[:],
                                         func=AF.Copy)
                nc.gpsimd.collective_compute(
                    "AllGather", ALU.bypass, replica_groups=[list(range(NCORES))],
                    ins=[xl_loc[l][:]], outs=[xl_full[l][:]])

            edge_layer(1, 4)
            act_prev, act_next = act_next, act_prev
            xlxr_layer(2)
            edge_layer(2, 1)
            act_prev, act_next = act_next, act_prev
            xlxr_layer(3)
            edge_layer(3, 1)

            # ---------------- MLP head ----------------
            for j in range(NLOC // 512):
                sl = slice(j * 512, (j + 1) * 512)
                pm1 = ps_mm.tile([P, 512], FP, space="PSUM", name="pm1", tag="pmm")
                for k in range(2):
                    nc.tensor.matmul(out=pm1[:DIM, :], lhsT=Wm1_sb[:, k * DIM:(k + 1) * DIM],
                                     rhs=actT[k][:, sl], start=(k == 0), stop=False)
                nc.tensor.matmul(out=pm1[:DIM, :], lhsT=bm1_sb[:], rhs=ones_row[:],
                                 start=False, stop=True)
                nc.scalar.activation(out=m1T[:DIM, sl], in_=pm1[:DIM, :], func=AF.Relu)
            for j in range(NLOC // 512):
                sl = slice(j * 512, (j + 1) * 512)
                pm2 = ps_mm.tile([P, 512], FP, space="PSUM", name="pm2", tag="pmm")
                nc.tensor.matmul(out=pm2[:DIM, :], lhsT=Wm2_sb[:], rhs=m1T[:DIM, sl],
                                 start=True, stop=False)
                nc.tensor.matmul(out=pm2[:DIM, :], lhsT=bm2_sb[:], rhs=ones_row[:],
                                 start=False, stop=True)
                nc.scalar.activation(out=m2T[:DIM, sl], in_=pm2[:DIM, :], func=AF.Relu)
            for j in range(NLOC // 512):
                sl = slice(j * 512, (j + 1) * 512)
                py = ps_mm.tile([P, 512], FP, space="PSUM", name="py", tag="pmm")
                nc.tensor.matmul(out=py[:1, :], lhsT=Wm3_sb[:], rhs=m2T[:DIM, sl],
                                 start=True, stop=False)
                nc.tensor.matmul(out=py[:1, :], lhsT=bm3_sb[:], rhs=ones_row[:],
                                 start=False, stop=True)
                nc.scalar.activation(out=y_sb[:, sl], in_=py[:1, :], func=AF.Copy)
            nc.sync.dma_start(out=out[:], in_=y_sb[:])

    nc.compile()
    return nc


_BUILD_CACHE = {}


def _get_program(NT):
    if NT not in _BUILD_CACHE:
        _BUILD_CACHE[NT] = _build(NT)
    return _BUILD_CACHE[NT]


def kernel(**inputs) -> np.ndarray:
    global LAST_RESULTS
    ii = {k: np.asarray(v) for k, v in inputs.items()}
    assert ii["x"].shape == (N, F_IN)
    for l in (1, 2, 3):
        assert not np.any(ii[f"b{l}"]), "GAT bias assumed zero"

    src_col, dst_col, mt, NT = _prep_edges(np.asarray(ii["edge_index"], np.int64))
    iota = np.arange(P, dtype=BF_NP)
    iota_f = np.tile(iota[None, :], (P, 1))

    def bf(a):
        return np.asarray(a, np.float32).astype(BF_NP)

    common = dict(
        Win=bf(ii["Win"]), b_in=bf(ii["b_in"])[None, :],
        Wskip=bf(ii["Wskip"]), bskip=bf(ii["bskip"])[None, :],
        Wm1=bf(ii["Wm1"]), bm1=bf(ii["bm1"])[None, :],
        Wm2=bf(ii["Wm2"]), bm2=bf(ii["bm2"])[None, :],
        Wm3=bf(ii["Wm3"]), bm3=bf(ii["bm3"])[None, :],
        iota_f=iota_f,
    )
    for l in (1, 2, 3):
        common[f"WL{l}"] = bf(ii[f"Wl{l}"])
        common[f"WR{l}"] = bf(ii[f"Wr{l}"])
        attf = np.asarray(ii[f"att{l}"], np.float32).reshape(-1)
        common[f"svec{l}"] = np.tile(0.6 * attf[None, :], (P, 1)).astype(BF_NP)

    x = np.asarray(ii["x"], np.float32)
    in_maps = []
    for c in range(NCORES):
        m = dict(common)
        m["xT"] = np.ascontiguousarray(x[c * NLOC:(c + 1) * NLOC].T).astype(BF_NP)
        m["src_col"] = src_col[c]
        m["dst_col"] = dst_col[c]
        m["mt_host"] = np.ascontiguousarray(mt[c].reshape(NBLK, P, NT * P))
        in_maps.append(m)

    nc = _get_program(NT)
    res = run_bass_kernel_spmd(nc, in_maps, list(range(NCORES)),
                               trace=bool(os.environ.get("GAT_TRACE")))
    LAST_RESULTS = res
    return np.concatenate([res.results[c]["out"].reshape(-1) for c in range(NCORES)])


# revision 13
# speedup vs baseline: 1.1749x; 1.0366x over previous
"""Trainium2 Bass kernel for the 3-layer GATv2 network (nn_GAT_35940286333219).

Sharding: nodes contiguously across 8 cores (2048 each); edges partitioned by
destination so segment-softmax/scatter-add stay local; per-layer AllGather of
the source-side transformed features XL = act @ Wl; per-edge source rows via
one dma_gather per (block, layer).

v3: C=256 (512B rows) so a single dma_gather fetches a whole block's edge rows
(48 SWDGE instructions instead of 864 indirect DMAs); per-edge logits via the
Lrelu activation (alpha=0.2) directly instead of the abs decomposition with
extra weight columns; all per-tile DVE element-wise work batched into
per-block ops with broadcast APs; the transposed one-hot M^T precomputed on
the host and DMA'd instead of per-tile PE transposes + PSUM round trips.
"""
import os
import numpy as np
import ml_dtypes

import concourse.bacc as bacc
import concourse.bass as bass
import concourse.mybir as mybir
import concourse.tile as tile
from concourse.bass_utils import run_bass_kernel_spmd
from concourse.masks import make_identity

P = 128
N = 16384
NCORES = 8
NLOC = N // NCORES          # 2048
NBLK = NLOC // P            # 16
F_IN = 128
DIM = 64
HID = 256
C = 256                     # gathered row width (bf16 -> 512B, dma_gather aligned)
FP = mybir.dt.float32
BF = mybir.dt.bfloat16
I16 = mybir.dt.int16
AF = mybir.ActivationFunctionType
ALU = mybir.AluOpType
AX = mybir.AxisListType
BF_NP = ml_dtypes.bfloat16

LAST_RESULTS = None


def _prep_edges(edge_index):
    src = np.concatenate([edge_index[0], np.arange(N, dtype=np.int64)])
    dst = np.concatenate([edge_index[1], np.arange(N, dtype=np.int64)])
    order = np.argsort(dst, kind="stable")
    src_s, dst_s = src[order], dst[order]
    blk = dst_s // P
    bc = np.bincount(blk, minlength=NCORES * NBLK)
    NT = int(np.ceil(bc.max() / P))
    EB = NT * P
    src_pad = np.zeros((NCORES, NBLK, EB), dtype=np.int64)   # pad idx 0 (valid row)
    dst_pad = np.full((NCORES, NBLK, EB), P, dtype=np.float32)  # P = pad marker
    starts = np.concatenate([[0], np.cumsum(bc)])
    for g in range(NCORES * NBLK):
        c, b = divmod(g, NBLK)
        s, e = starts[g], starts[g + 1]
        k = e - s
        src_pad[c, b, :k] = src_s[s:e]
        dst_pad[c, b, :k] = (dst_s[s:e] - g * P).astype(np.float32)
    # dst one-hot transposed [d, i]: column i = edge i (partition p=i%128, tile t=i//128)
    mt = (dst_pad[:, :, None, :] == np.arange(P, dtype=np.float32)[None, None, :, None])
    mt = mt.astype(BF_NP)                                     # [NC, NBLK, 128, EB]
    # dst_col [NC, NBLK, P, NT]: value of edge i=t*P+p at [p, t]
    dst_col = dst_pad.reshape(NCORES, NBLK, NT, P).transpose(0, 1, 3, 2)
    dst_col = np.ascontiguousarray(dst_col).astype(BF_NP)
    # gather offsets [p, t]: row index of edge i = t*P + p
    src_col = src_pad.reshape(NCORES, NBLK, NT, P).transpose(0, 1, 3, 2)
    src_col = np.ascontiguousarray(src_col).astype(np.int32)
    return src_col, dst_col, mt, NT


def _build(NT):
    nc = bacc.Bacc(None)

    def par(name, shape, dtype=BF):
        return nc.declare_dram_parameter(name, list(shape), dtype, isOutput=False)

    xT = par("xT", [F_IN, NLOC])
    src_col = par("src_col", [NBLK, P, NT], mybir.dt.int32)
    dst_col = par("dst_col", [NBLK, P, NT], BF)
    mt_host = par("mt_host", [NBLK, P, NT * P], BF)
    CL = {1: 260, 2: 258, 3: 258}
    Win = par("Win", [F_IN, DIM]); b_in = par("b_in", [1, DIM])
    Wskip = par("Wskip", [DIM, HID]); bskip = par("bskip", [1, HID])
    WL1 = par("WL1", [DIM, CL[1]]); WR1 = par("WR1", [DIM, C])
    WL2 = par("WL2", [HID, CL[2]]); WR2 = par("WR2", [HID, C])
    WL3 = par("WL3", [HID, CL[3]]); WR3 = par("WR3", [HID, C])
    svec1 = par("svec1", [P, C]); svec2 = par("svec2", [P, C]); svec3 = par("svec3", [P, C])
    Wm1 = par("Wm1", [HID, DIM]); bm1 = par("bm1", [1, DIM])
    Wm2 = par("Wm2", [DIM, DIM]); bm2 = par("bm2", [1, DIM])
    Wm3 = par("Wm3", [DIM, 1]); bm3 = par("bm3", [1, 1])
    iota_f = par("iota_f", [P, P])
    out = nc.declare_dram_parameter("out", [1, NLOC], FP, isOutput=True)

    xl_loc = {l: nc.dram_tensor(f"xl_loc{l}", [NLOC, CL[l]], BF) for l in (1, 2, 3)}
    xl_full = {l: nc.dram_tensor(f"xl_full{l}", [N, CL[l]], BF, addr_space="Shared")
               for l in (1, 2, 3)}

    with tile.TileContext(nc) as tc:
        with (
            tc.tile_pool(name="const", bufs=1) as cp,
            tc.tile_pool(name="big", bufs=1) as bigp,
            tc.tile_pool(name="wk", bufs=1) as wk,
            tc.tile_pool(name="ps_mm", bufs=2, space="PSUM") as ps_mm,
            tc.tile_pool(name="ps_out", bufs=2, space="PSUM") as ps_out_pool,
            tc.tile_pool(name="ps_w", bufs=2, space="PSUM") as ps_w_pool,
        ):
            def load_const(pname, ap, shape, dtype=BF):
                t = cp.tile(list(shape), dtype, name=pname + "_sb")
                nc.sync.dma_start(out=t[:], in_=ap[:])
                return t

            def load_const_2k(pname, ap, rows, cols):
                assert rows == 2 * P
                t = cp.tile([P, 2 * cols], BF, name=pname + "_sb")
                nc.sync.dma_start(out=t[:, :cols], in_=ap[:P, :])
                nc.sync.dma_start(out=t[:, cols:], in_=ap[P:, :])
                return t

            ident_f = cp.tile([P, P], FP, name="ident_f")
            make_identity(nc, ident_f[:])
            ident_b = cp.tile([P, P], BF, name="ident_b")
            nc.vector.tensor_copy(out=ident_b[:], in_=ident_f[:])
            ones_row = cp.tile([1, 512], BF, name="ones_row")
            nc.vector.memset(ones_row[:], 1.0)

            xT_sb = load_const("xT", xT, [F_IN, NLOC])
            Win_sb = load_const("Win", Win, [F_IN, DIM])
            b_in_sb = load_const("b_in", b_in, [1, DIM])
            Wskip_sb = load_const("Wskip", Wskip, [DIM, HID])
            bskip_sb = load_const("bskip", bskip, [1, HID])
            WL_sb = {1: load_const("WL1", WL1, [DIM, CL[1]]),
                     2: load_const_2k("WL2", WL2, HID, CL[2]),
                     3: load_const_2k("WL3", WL3, HID, CL[3])}
            WR_sb = {1: load_const("WR1", WR1, [DIM, C]),
                     2: load_const_2k("WR2", WR2, HID, C),
                     3: load_const_2k("WR3", WR3, HID, C)}
            svec_sb = {1: load_const("svec1", svec1, [P, C]),
                       2: load_const("svec2", svec2, [P, C]),
                       3: load_const("svec3", svec3, [P, C])}
            Wm1_sb = load_const_2k("Wm1", Wm1, HID, DIM)
            bm1_sb = load_const("bm1", bm1, [1, DIM])
            Wm2_sb = load_const("Wm2", Wm2, [DIM, DIM])
            bm2_sb = load_const("bm2", bm2, [1, DIM])
            Wm3_sb = load_const("Wm3", Wm3, [DIM, 1])
            bm3_sb = load_const("bm3", bm3, [1, 1])
            iof_sb = load_const("iota_f", iota_f, [P, P])

            actT = {0: bigp.tile([P, NLOC], BF, name="actT0"),
                    1: bigp.tile([P, NLOC], BF, name="actT1")}
            act_prev = bigp.tile([P, NBLK * HID], FP, name="act_prev")
            act_next = bigp.tile([P, NBLK * HID], FP, name="act_next")
            XRb = bigp.tile([P, NBLK * C], BF, name="XRb")
            hT = bigp.tile([DIM, NLOC], BF, name="hT")
            m1T = bigp.tile([DIM, NLOC], BF, name="m1T")
            m2T = bigp.tile([DIM, NLOC], BF, name="m2T")
            y_sb = bigp.tile([1, NLOC], FP, name="y_sb")

            # ---------------- phase A ----------------
            for j in range(NLOC // 512):
                sl = slice(j * 512, (j + 1) * 512)
                pmm = ps_mm.tile([P, 512], FP, space="PSUM", name="pmm", tag="pmm")
                nc.tensor.matmul(out=pmm[:DIM, :], lhsT=Win_sb[:], rhs=xT_sb[:, sl],
                                 start=True, stop=False)
                nc.tensor.matmul(out=pmm[:DIM, :], lhsT=b_in_sb[:], rhs=ones_row[:],
                                 start=False, stop=True)
                nc.scalar.activation(out=hT[:DIM, sl], in_=pmm[:DIM, :], func=AF.Relu)

            for b in range(NBLK):
                nsl = slice(b * P, (b + 1) * P)
                pxl = ps_mm.tile([P, CL[1]], FP, space="PSUM", name="pxl", tag="pmm")
                nc.tensor.matmul(out=pxl[:], lhsT=hT[:DIM, nsl], rhs=WL_sb[1][:],
                                 start=True, stop=True)
                xl_st = wk.tile([P, CL[1]], BF, name="xl_st", tag="xl_st", bufs=3)
                nc.scalar.activation(out=xl_st[:], in_=pxl[:], func=AF.Copy)
                nc.sync.dma_start(out=xl_loc[1][nsl, :], in_=xl_st[:])

                pxr = ps_mm.tile([P, C], FP, space="PSUM", name="pxr", tag="pmm")
                nc.tensor.matmul(out=pxr[:], lhsT=hT[:DIM, nsl], rhs=WR_sb[1][:],
                                 start=True, stop=True)
                nc.scalar.activation(out=XRb[:, b * C:(b + 1) * C], in_=pxr[:], func=AF.Copy)

                psk = ps_mm.tile([P, HID], FP, space="PSUM", name="psk", tag="pmm")
                nc.tensor.matmul(out=psk[:], lhsT=hT[:DIM, nsl], rhs=Wskip_sb[:],
                                 start=True, stop=False)
                nc.tensor.matmul(out=psk[:], lhsT=ones_row[:, :P], rhs=bskip_sb[:],
                                 start=False, stop=True)
                nc.scalar.activation(out=act_prev[:, b * HID:(b + 1) * HID], in_=psk[:],
                                     func=AF.Copy)

            nc.gpsimd.collective_compute(
                "AllGather", ALU.bypass, replica_groups=[list(range(NCORES))],
                ins=[xl_loc[1][:]], outs=[xl_full[1][:]])

            # ---------------- edge stage ----------------
            svec_rep = bigp.tile([P, NT * C], BF, name="svec_rep")

            def edge_layer(l, H):
                D = C // H
                CT = CL[l]                      # gathered row: [xl(256) | (1) | sl(H)]
                off0 = 256 if H > 1 else 257    # sl column offset within a row
                nc.vector.tensor_copy(
                    out=svec_rep[:].rearrange("p (t c) -> p t c", t=NT),
                    in_=svec_sb[l][:, None, :].to_broadcast([P, NT, C]))
                for b in range(NBLK):
                    src_b = wk.tile([P, NT], mybir.dt.int32, name="src_b",
                                    tag="src_b", bufs=2)
                    nc.sync.dma_start(out=src_b[:], in_=src_col[b])
                    dst_b = wk.tile([P, NT], BF, name="dst_b", tag="dst_b", bufs=2)
                    nc.sync.dma_start(out=dst_b[:], in_=dst_col[b])
                    mt_b = wk.tile([P, NT * P], BF, name="mt_b", tag="mt_b", bufs=2)
                    nc.sync.dma_start(out=mt_b[:], in_=mt_host[b])
                    m_b = wk.tile([P, NT * P], BF, name="m_b", tag="m_b", bufs=2)
                    nc.vector.tensor_tensor(
                        out=m_b[:].rearrange("p (t e) -> p t e", t=NT),
                        in0=dst_b[:, :, None].to_broadcast([P, NT, P]),
                        in1=iof_sb[:, None, :].to_broadcast([P, NT, P]),
                        op=ALU.is_equal)
                    xl_all = wk.tile([P, NT * CT], BF, name="xl_all", tag="xl_all", bufs=3)
                    for t in range(NT):
                        nc.gpsimd.indirect_dma_start(
                            out=xl_all[:, t * CT:(t + 1) * CT], out_offset=None,
                            in_=xl_full[l][:],
                            in_offset=bass.IndirectOffsetOnAxis(
                                ap=src_b[:, t:t + 1], axis=0))
                    # w = M @ XR + xl  (PE); P_all = |w| (ACT).  Logits:
                    # att.lrelu(w) = 0.6 att.w + 0.4 att.|w|; the xr half of the
                    # linear term is constant per dst and cancels in the
                    # softmax, the xl half rides the table as sl columns.
                    P_all = wk.tile([P, NT * C], BF, name="P_all", tag="P_all", bufs=2)
                    for t in range(NT):
                        w_ps = ps_w_pool.tile([P, C], FP, space="PSUM",
                                              name="w_ps", tag="w_ps")
                        nc.tensor.matmul(out=w_ps[:], lhsT=mt_b[:, t * P:(t + 1) * P],
                                         rhs=XRb[:, b * C:(b + 1) * C],
                                         start=True, stop=False)
                        nc.tensor.matmul(out=w_ps[:], lhsT=ident_b[:],
                                         rhs=xl_all[:, t * CT:t * CT + C],
                                         start=False, stop=True)
                        nc.scalar.activation(out=P_all[:, t * C:(t + 1) * C],
                                             in_=w_ps[:], func=AF.Abs)
                    nc.vector.tensor_tensor(out=P_all[:], in0=P_all[:],
                                            in1=svec_rep[:], op=ALU.mult)
                    r_all = wk.tile([P, NT * H], FP, name="r_all", tag="r_all", bufs=2)
                    nc.vector.tensor_reduce(
                        out=r_all[:].rearrange("p (t h) -> p t h", t=NT),
                        in_=P_all[:].rearrange("p (t h d) -> p t h d", t=NT, h=H),
                        axis=AX.X, op=ALU.add)
                    nc.vector.tensor_tensor(
                        out=r_all[:].rearrange("p (t h) -> p t h", t=NT),
                        in0=r_all[:].rearrange("p (t h) -> p t h", t=NT),
                        in1=xl_all[:].rearrange("p (t ct) -> p t ct", ct=CT)[:, :, off0:off0 + H],
                        op=ALU.add)
                    alpha_all = wk.tile([P, NT * H], BF, name="alpha_all",
                                        tag="alpha_all", bufs=2)
                    nc.scalar.activation(out=alpha_all[:], in_=r_all[:], func=AF.Exp)
                    ps_o = ps_out_pool.tile([P, C + 4], FP, space="PSUM",
                                            name="ps_o", tag="ps_o")
                    if H == 1:
                        # scale the one-hot M by alpha; denominator via the
                        # table's ones column (col 256) streamed with xl
                        mp_all = wk.tile([P, NT * P], BF, name="mp_all",
                                         tag="mp_all", bufs=2)
                        nc.vector.tensor_tensor(
                            out=mp_all[:].rearrange("p (t e) -> p t e", t=NT),
                            in0=m_b[:].rearrange("p (t e) -> p t e", t=NT),
                            in1=alpha_all[:, :, None].to_broadcast([P, NT, P]),
                            op=ALU.mult)
                        for t in range(NT):
                            nc.tensor.matmul(out=ps_o[:, :C + 1],
                                             lhsT=mp_all[:, t * P:(t + 1) * P],
                                             rhs=xl_all[:, t * CT:t * CT + C + 1],
                                             start=(t == 0), stop=(t == NT - 1))
                    else:
                        CV = C + H
                        v_all = wk.tile([P, NT * CV], BF, name="v_all",
                                        tag="v_all", bufs=2)
                        v_r = v_all[:].rearrange("p (t cv) -> p t cv", t=NT)
                        nc.vector.tensor_tensor(
                            out=v_r[:, :, :C].rearrange("p t (h d) -> p t h d", h=H),
                            in0=xl_all[:].rearrange("p (t ct) -> p t ct", ct=CT)
                                [:, :, :C].rearrange("p t (h d) -> p t h d", h=H),
                            in1=alpha_all[:].rearrange("p (t h) -> p t h", t=NT)
                                [:, :, :, None].to_broadcast([P, NT, H, D]),
                            op=ALU.mult)
                        nc.vector.tensor_copy(
                            out=v_r[:, :, C:],
                            in_=alpha_all[:].rearrange("p (t h) -> p t h", t=NT))
                        for t in range(NT):
                            nc.tensor.matmul(out=ps_o[:, :CV],
                                             lhsT=m_b[:, t * P:(t + 1) * P],
                                             rhs=v_all[:, t * CV:(t + 1) * CV],
                                             start=(t == 0), stop=(t == NT - 1))
                    # finalize: normalize, relu, residual, transpose
                    rec = wk.tile([P, H], FP, name="rec", tag="rec", bufs=2)
                    nc.vector.reciprocal(out=rec[:], in_=ps_o[:, C:C + H])
                    g_t = wk.tile([P, C], FP, name="g_t", tag="g_t", bufs=2)
                    nc.vector.tensor_tensor(
                        out=g_t[:].rearrange("p (h d) -> p h d", h=H),
                        in0=ps_o[:, :C].rearrange("p (h d) -> p h d", h=H),
                        in1=rec[:, :, None].to_broadcast([P, H, D]),
                        op=ALU.mult)
                    gr = wk.tile([P, C], FP, name="gr", tag="gr", bufs=2)
                    nc.scalar.activation(out=gr[:], in_=g_t[:], func=AF.Relu)
                    bsl = slice(b * HID, (b + 1) * HID)
                    nc.vector.tensor_tensor(out=act_next[:, bsl], in0=gr[:],
                                            in1=act_prev[:, bsl], op=ALU.add)
                    for k in range(2):
                        tp = ps_mm.tile([P, P], FP, space="PSUM", name="tp", tag="pmm")
                        nc.tensor.transpose(
                            out=tp[:], in_=act_next[:, b * HID + k * P:b * HID + (k + 1) * P],
                            identity=ident_f[:])
                        nc.scalar.activation(out=actT[k][:, b * P:(b + 1) * P],
                                             in_=tp[:], func=AF.Copy)

            _ocr = {}

            def _ones_col_row(l):
                if l not in _ocr:
                    t = cp.tile([1, CL[l]], BF, name=f"onescol{l}")
                    nc.vector.memset(t[:], 0.0)
                    nc.vector.memset(t[:, 256:257], 1.0)
                    _ocr[l] = t
                return _ocr[l][:]

            def xlxr_layer(l):
                CT = CL[l]
                for b in range(NBLK):
                    nsl = slice(b * P, (b + 1) * P)
                    pxl = ps_mm.tile([P, CT], FP, space="PSUM", name="pxl2", tag="pmm")
                    for k in range(2):
                        nc.tensor.matmul(out=pxl[:], lhsT=actT[k][:, nsl],
                                         rhs=WL_sb[l][:, k * CT:(k + 1) * CT],
                                         start=(k == 0), stop=False)
                    nc.tensor.matmul(out=pxl[:], lhsT=ones_row[:, :P],
                                     rhs=_ones_col_row(l), start=False, stop=True)
                    xl_st = wk.tile([P, CT], BF, name="xl_st2", tag="xl_st", bufs=3)
                    nc.scalar.activation(out=xl_st[:], in_=pxl[:], func=AF.Copy)
                    nc.sync.dma_start(out=xl_loc[l][nsl, :], in_=xl_st[:])

                    pxr = ps_mm.tile([P, C], FP, space="PSUM", name="pxr2", tag="pmm")
                    for k in range(2):
                        nc.tensor.matmul(out=pxr[:], lhsT=actT[k][:, nsl],
                                         rhs=WR_sb[l][:, k * C:(k + 1) * C],
                                         start=(k == 0), stop=(k == 1))
                    nc.scalar.activation(out=XRb[:, b * C:(b + 1) * C], in_=pxr[:],
                                         func=AF.Copy)
                nc.gpsimd.collective_compute(
                    "AllGather", ALU.bypass, replica_groups=[list(range(NCORES))],
                    ins=[xl_loc[l][:]], outs=[xl_full[l][:]])

            edge_layer(1, 4)
            act_prev, act_next = act_next, act_prev
            xlxr_layer(2)
            edge_layer(2, 1)
            act_prev, act_next = act_next, act_prev
            xlxr_layer(3)
            edge_layer(3, 1)

            # ---------------- MLP head ----------------
            for j in range(NLOC // 512):
                sl = slice(j * 512, (j + 1) * 512)
                pm1 = ps_mm.tile([P, 512], FP, space="PSUM", name="pm1", tag="pmm")
                for k in range(2):
                    nc.tensor.matmul(out=pm1[:DIM, :], lhsT=Wm1_sb[:, k * DIM:(k + 1) * DIM],
                                     rhs=actT[k][:, sl], start=(k == 0), stop=False)
                nc.tensor.matmul(out=pm1[:DIM, :], lhsT=bm1_sb[:], rhs=ones_row[:],
                                 start=False, stop=True)
                nc.scalar.activation(out=m1T[:DIM, sl], in_=pm1[:DIM, :], func=AF.Relu)
            for j in range(NLOC // 512):
                sl = slice(j * 512, (j + 1) * 512)
                pm2 = ps_mm.tile([P, 512], FP, space="PSUM", name="pm2", tag="pmm")
                nc.tensor.matmul(out=pm2[:DIM, :], lhsT=Wm2_sb[:], rhs=m1T[:DIM, sl],
                                 start=True, stop=False)
                nc.tensor.matmul(out=pm2[:DIM, :], lhsT=bm2_sb[:], rhs=ones_row[:],
                                 start=False, stop=True)
                nc.scalar.activation(out=m2T[:DIM, sl], in_=pm2[:DIM, :], func=AF.Relu)
            for j in range(NLOC // 512):
                sl = slice(j * 512, (j + 1) * 512)
                py = ps_mm.tile([P, 512], FP, space="PSUM", name="py", tag="pmm")
                nc.tensor.matmul(out=py[:1, :], lhsT=Wm3_sb[:], rhs=m2T[:DIM, sl],
                                 start=True, stop=False)
                nc.tensor.matmul(out=py[:1, :], lhsT=bm3_sb[:], rhs=ones_row[:],
                                 start=False, stop=True)
                nc.scalar.activation(out=y_sb[:, sl], in_=py[:1, :], func=AF.Copy)
            nc.sync.dma_start(out=out[:], in_=y_sb[:])

    nc.compile()
    return nc


_BUILD_CACHE = {}


def _get_program(NT):
    if NT not in _BUILD_CACHE:
        _BUILD_CACHE[NT] = _build(NT)
    return _BUILD_CACHE[NT]


def kernel(**inputs) -> np.ndarray:
    global LAST_RESULTS
    ii = {k: np.asarray(v) for k, v in inputs.items()}
    assert ii["x"].shape == (N, F_IN)
    for l in (1, 2, 3):
        assert not np.any(ii[f"b{l}"]), "GAT bias assumed zero"

    src_col, dst_col, mt, NT = _prep_edges(np.asarray(ii["edge_index"], np.int64))
    iota = np.arange(P, dtype=BF_NP)
    iota_f = np.tile(iota[None, :], (P, 1))

    def bf(a):
        return np.asarray(a, np.float32).astype(BF_NP)

    common = dict(
        Win=bf(ii["Win"]), b_in=bf(ii["b_in"])[None, :],
        Wskip=bf(ii["Wskip"]), bskip=bf(ii["bskip"])[None, :],
        Wm1=bf(ii["Wm1"]), bm1=bf(ii["bm1"])[None, :],
        Wm2=bf(ii["Wm2"]), bm2=bf(ii["bm2"])[None, :],
        Wm3=bf(ii["Wm3"]), bm3=bf(ii["bm3"])[None, :],
        iota_f=iota_f,
    )
    for l, H in ((1, 4), (2, 1), (3, 1)):
        Wl = np.asarray(ii[f"Wl{l}"], np.float32)
        att = np.asarray(ii[f"att{l}"], np.float32)
        inD = Wl.shape[0]
        D = HID // H
        # per-head source-side linear logit term: sl = 0.6 * att . xl
        Wsl = 0.6 * np.stack([(Wl[:, h * D:(h + 1) * D] * att[h][None, :]).sum(1)
                              for h in range(H)], axis=1)
        if H == 1:
            WL = np.concatenate([Wl, np.zeros((inD, 1), np.float32), Wsl], 1)
        else:
            WL = np.concatenate([Wl, Wsl], 1)
        common[f"WL{l}"] = WL.astype(BF_NP)
        common[f"WR{l}"] = bf(ii[f"Wr{l}"])
        attf = att.reshape(-1)
        common[f"svec{l}"] = np.tile(0.4 * attf[None, :], (P, 1)).astype(BF_NP)

    x = np.asarray(ii["x"], np.float32)
    in_maps = []
    for c in range(NCORES):
        m = dict(common)
        m["xT"] = np.ascontiguousarray(x[c * NLOC:(c + 1) * NLOC].T).astype(BF_NP)
        m["src_col"] = src_col[c]
        m["dst_col"] = dst_col[c]
        m["mt_host"] = np.ascontiguousarray(mt[c].reshape(NBLK, P, NT * P))
        in_maps.append(m)

    nc = _get_program(NT)
    res = run_bass_kernel_spmd(nc, in_maps, list(range(NCORES)),
                               trace=bool(os.environ.get("GAT_TRACE")))
    LAST_RESULTS = res
    return np.concatenate([res.results[c]["out"].reshape(-1) for c in range(NCORES)])
